# revision 34
# baseline (speedup 1.0000x reference)
"""AttentionConv2d Trainium2 kernel — 8-core batch-data-parallel (v3).

Each of the 8 NeuronCores processes one image of the batch:
  - qkv 1x1 conv (q,k only) + 3x3 conv as implicit-GEMM matmuls
  - V projection computed transposed (x as stationary) so the A*V stationary
    needs no on-device transposes
  - relative position logits G[y2,l]/W[x2,l] produced by 64 diagonal-packed
    matmuls (4 heads x 2 j-halves per pass via block-diagonal kr windows
    built on host)
  - per-head logits via contract-dim augmentation (32 k + 32 G-sel + 32 W-sel
    rows); logits/AV matmuls interleaved mb-wise to keep PE busy (p-state)
  - softmax exp on ScalarE from PSUM; denominators ride the A*V matmul as an
    appended ones-column; fast approx reciprocal + deferred division pipeline
  - final 1x1 conv; outputs concatenated [conv_out(256) ; attn(256)]
  - padded-image build + 3x3 conv run off the attention critical path
"""

import os
import sys

import numpy as np
import ml_dtypes

sys.path.insert(0, "/opt/trn_rl_repo")

B, C_IN, H, W = 8, 256, 32, 32
HW = H * W
DK = DV = 256
NH = 8
DKH = DK // NH  # 32
C_OUT = 512
N_CORES = 8

_CACHE = {}


def _build():
    import concourse.bass as bass
    import concourse.mybir as mybir
    import concourse.tile as tile
    from concourse import bacc
    from contextlib import ExitStack

    f32 = mybir.dt.float32
    bf16 = mybir.dt.bfloat16
    AF = mybir.ActivationFunctionType

    nc = bacc.Bacc("TRN2", target_bir_lowering=False, debug=False,
                   num_devices=N_CORES)

    x_d = nc.dram_tensor("x", [2, 128, HW], f32, kind="ExternalInput").ap()
    wqkv_d = nc.dram_tensor("wqkvT", [2, 128, 768], bf16, kind="ExternalInput").ap()
    wout_d = nc.dram_tensor("woutT", [2, 128, 9, 256], bf16, kind="ExternalInput").ap()
    wattn_d = nc.dram_tensor("wattnT", [2, 128, 256], bf16, kind="ExternalInput").ap()
    khd_d = nc.dram_tensor("krhdiag", [128, 32, 128], bf16, kind="ExternalInput").ap()
    kwd_d = nc.dram_tensor("krwdiag", [128, 32, 128], bf16, kind="ExternalInput").ap()
    masks_d = nc.dram_tensor("masks", [64, 8, 128], bf16, kind="ExternalInput").ap()
    bqkv_d = nc.dram_tensor("bqkv", [128, 4], f32, kind="ExternalInput").ap()
    batt_d = nc.dram_tensor("battn", [128, 2], f32, kind="ExternalInput").ap()
    bout_d = nc.dram_tensor("bout", [128, 2], f32, kind="ExternalInput").ap()
    out_d = nc.dram_tensor("out", [4, 128, HW], f32, kind="ExternalOutput").ap()

    with tile.TileContext(nc) as tc, ExitStack() as ctx:
        wp = ctx.enter_context(tc.tile_pool(name="weights", bufs=1))
        ap_ = ctx.enter_context(tc.tile_pool(name="acts", bufs=1))
        hp = ctx.enter_context(tc.tile_pool(name="head", bufs=2))
        pbig = ctx.enter_context(tc.tile_pool(name="pbig", bufs=2, space="PSUM"))

        # ---- weights / constants to SBUF ----
        wqkv = wp.tile([128, 2, 768], bf16)
        wout = wp.tile([128, 2, 9, 256], bf16)
        wattn = wp.tile([128, 2, 256], bf16)
        khd = wp.tile([128, 32, 128], bf16)
        kwd = wp.tile([128, 32, 128], bf16)
        tmpl = wp.tile([96, 2, 8, 128], bf16)  # [contract, k-slot, mb, jj]
        bqkv = wp.tile([128, 4], f32)
        batt = wp.tile([128, 2], f32)
        bout = wp.tile([128, 2], f32)

        # ---- x first (critical path); fan DMA issue across idle engines ----
        x32 = ap_.tile([128, 2, HW], f32)
        xc = ap_.tile([128, 2, HW], bf16)   # contiguous bf16 image
        xeng = [nc.sync, nc.scalar]
        for j in range(2):
            for hh in range(2):
                xeng[j].dma_start(x32[:, j, hh * 512:(hh + 1) * 512],
                                  x_d[j, :, hh * 512:(hh + 1) * 512])
        nc.sync.dma_start(bqkv[:], bqkv_d[:])
        for j in range(2):
            xeng[j].dma_start(wqkv[:, j, :], wqkv_d[j])
        for hh in range(2):
            nc.scalar.dma_start(khd[:, 16 * hh:16 * hh + 16, :],
                                khd_d[:, 16 * hh:16 * hh + 16, :])
            nc.scalar.dma_start(kwd[:, 16 * hh:16 * hh + 16, :],
                                kwd_d[:, 16 * hh:16 * hh + 16, :])
        for s in range(2):
            nc.gpsimd.dma_start(tmpl[32:96, s, :, :], masks_d[:])
        ceng = [nc.vector, nc.gpsimd]
        for hh in range(2):
            for j in range(2):
                ceng[j].tensor_copy(
                    xc[:, j, hh * 512:(hh + 1) * 512],
                    x32[:, j, hh * 512:(hh + 1) * 512])
        for j in range(2):
            nc.sync.dma_start(wattn[:, j, :], wattn_d[j])
            nc.sync.dma_start(wout[:, j, :, :], wout_d[j])
        nc.sync.dma_start(batt[:], batt_d[:])
        nc.sync.dma_start(bout[:], bout_d[:])

        # ---- qkv = Wqkv @ x (1x1 conv), q then k; vT interleaved ----
        qkv = ap_.tile([128, 4, 32, 32], bf16)
        vTe = ap_.tile([128, 8, 8, 33], bf16)  # [m, mb, h, d(+ones)]
        nc.gpsimd.memset(vTe[:], 1.0)

        def qkv_ob(ob):
            ps = pbig.tile([128, HW], f32, tag="big", name=f"qkvps{ob}")
            for half in range(2):
                for j in range(2):
                    nc.tensor.matmul(
                        ps[:, half * 512:(half + 1) * 512],
                        wqkv[:, j, ob * 128:(ob + 1) * 128],
                        xc[:, j, half * 512:(half + 1) * 512],
                        start=(j == 0), stop=(j == 1),
                    )
            dst = qkv[:, ob, :, :].rearrange("p y x -> p (y x)")
            if ob < 2:
                nc.vector.tensor_scalar_add(dst, ps[:], bqkv[:, ob:ob + 1])
            else:
                nc.scalar.activation(dst, ps[:], AF.Identity,
                                     bias=bqkv[:, ob:ob + 1])

        def vT_half(hb):  # m-blocks 4*hb .. 4*hb+3
            pv = pbig.tile([128, HW], f32, tag="big")
            for bb in range(4):
                b = 4 * hb + bb
                for j in range(2):
                    nc.tensor.matmul(
                        pv[:, bb * 256:(bb + 1) * 256],
                        xc[:, j, 128 * b:128 * (b + 1)],
                        wqkv[:, j, 512:768],
                        start=(j == 0), stop=(j == 1),
                    )
            nc.vector.tensor_copy(
                vTe[:, 4 * hb:4 * hb + 4, :, 0:32],
                pv.rearrange("p (b h d) -> p b h d", b=4, h=8, d=32),
            )

        qkv_ob(0)
        qkv_ob(1)
        vT_half(0)
        vT_half(1)
        qkv_ob(2)
        qkv_ob(3)

        # ---- rel-position logits, 4 heads x 2 j per pass ----
        # G[32i+y2, j, y, x] = sum_d krh[31+y2-y, d] * q[32i+d, j, y, x]
        grelB = ap_.tile([128, 2, HW], bf16)
        wrelB = ap_.tile([128, 2, HW], bf16)
        rhs0 = hp.tile([96, HW], bf16, tag="rhs")
        with tc.tile_pool(name="pmx", bufs=1, space="PSUM") as pmx:
            pg = pmx.tile([128, 2048], f32)
            for y in range(32):
                nc.tensor.matmul(
                    pg[:, y * 64:(y + 1) * 64], khd[:, y, :], qkv[:, 0:2, y, :],
                    start=True, stop=True,
                )
            # head 0's G rows straight from PSUM (skips the staging wait)
            nc.vector.tensor_copy(
                rhs0[32:64, :].rearrange("p (y x) -> p y x", y=32, x=32),
                pg.rearrange("p (y j x) -> p y j x", y=32, j=2, x=32)[0:32, :, 0, :],
            )
            # pg cols are (y, j, x) -> reorder to (j, y, x) during staging
            nc.scalar.activation(
                grelB[:],
                pg.rearrange("p (y j x) -> p j y x", y=32, j=2, x=32),
                AF.Copy)
            # W-side into pbig tiles so it does not wait on grelB staging
            for xh in range(2):
                pw = pbig.tile([128, HW], f32, tag="big")
                for xx in range(16):
                    x = 16 * xh + xx
                    nc.tensor.matmul(
                        pw[:, xx * 64:(xx + 1) * 64], kwd[:, x, :],
                        qkv[:, 0:2, :, x],
                        start=True, stop=True,
                    )
                # head 0's W rows for this x-half straight from PSUM
                nc.vector.tensor_copy(
                    rhs0[64:96, :].rearrange("p (y x) -> p x y", y=32, x=32)
                    [:, 16 * xh:16 * xh + 16, :],
                    pw.rearrange("p (x j y) -> p x j y", x=16, j=2, y=32)
                    [0:32, :, 0, :],
                )
                # pw cols are (x, j, y) -> (j, y, x)
                nc.scalar.activation(
                    wrelB[:].rearrange("p j (y x) -> p j y x", y=32, x=32)
                    [:, :, :, 16 * xh:16 * xh + 16],
                    pw.rearrange("p (x j y) -> p j y x", x=16, j=2, y=32),
                    AF.Copy)

        # ---- per-head attention ----
        pav = ctx.enter_context(tc.tile_pool(name="pav", bufs=2, space="PSUM"))
        attn = ap_.tile([128, 2, HW], bf16)
        pend = []

        def divide(avp, hp0, j):
            dn = hp.tile([1, HW], f32, tag="dn")
            nc.vector.tensor_copy(dn[:], avp[32:33, :])
            rdn = hp.tile([1, HW], f32, tag="rdn")
            nc.vector.reciprocal_approx_fast(rdn[:], dn[:])
            rb = hp.tile([32, HW], f32, tag="rb")
            nc.gpsimd.partition_broadcast(rb[:], rdn[:])
            nc.vector.tensor_mul(attn[hp0:hp0 + 32, j, :], avp[0:32, :], rb[:])

        def av_mb(ctx_h, mb):
            st, avp, hh = ctx_h
            for lh in range(2):
                nc.tensor.matmul(
                    avp[:, lh * 512:(lh + 1) * 512],
                    vTe[:, mb, hh, :],
                    st[:, mb, lh * 512:(lh + 1) * 512],
                    start=(mb == 0), stop=(mb == 7),
                )

        prev = None  # (st, avp, h) with A6/A7 still to issue
        for h in range(NH):
            i, j, s = h % 4, h // 4, h % 2
            hp0 = 32 * i
            nc.vector.tensor_copy(
                tmpl[0:32, s, :, :],
                qkv[hp0:hp0 + 32, 2 + j, :, :].rearrange("p y x -> p (y x)")
                .rearrange("p (m c) -> p m c", m=8, c=128),
            )
            if h == 0:
                rhs = rhs0
                nc.vector.tensor_copy(
                    rhs[0:32, :],
                    qkv[hp0:hp0 + 32, j, :, :].rearrange("p y x -> p (y x)"))
            else:
                rhs = hp.tile([96, HW], bf16, tag="rhs")
                nc.vector.tensor_copy(
                    rhs[0:32, :],
                    qkv[hp0:hp0 + 32, j, :, :].rearrange("p y x -> p (y x)"))
                nc.vector.tensor_copy(rhs[32:64, :], grelB[hp0:hp0 + 32, j, :])
                nc.vector.tensor_copy(rhs[64:96, :], wrelB[hp0:hp0 + 32, j, :])

            st = hp.tile([128, 8, HW], bf16, tag="st")
            avp = pav.tile([33, HW], f32, tag="av")
            cur = (st, avp, h)

            def logits_mb(mb):
                ps = pbig.tile([128, HW], f32, tag="big")
                for lh in range(2):
                    nc.tensor.matmul(
                        ps[:, lh * 512:(lh + 1) * 512],
                        tmpl[:, s, mb, :],
                        rhs[:, lh * 512:(lh + 1) * 512],
                        start=True, stop=True,
                    )
                nc.scalar.activation(st[:, mb, :], ps[:], AF.Exp)

            # cross-head pipeline: finish the previous head's tail A*V inside
            # this head's slot stream, then its division, so PE never drains
            logits_mb(0)
            logits_mb(1)
            if prev is not None:
                av_mb(prev, 6)
                av_mb(prev, 7)
                divide(*pend.pop())
            for mb in range(2, 8):
                av_mb(cur, mb - 2)
                logits_mb(mb)
            pend.append((avp, hp0, j))
            prev = cur

        av_mb(prev, 6)
        av_mb(prev, 7)
        if pend:
            divide(*pend.pop())

        # ---- padded image for 3x3 conv (off critical path) ----
        xp = ap_.tile([128, 2, 34 * 34], bf16)
        nc.gpsimd.memset(xp[:], 0.0)
        for j in range(2):
            nc.gpsimd.tensor_copy(
                xp[:, j, :].rearrange("p (y x) -> p y x", y=34, x=34)[:, 1:33, 1:33],
                xc[:, j, :].rearrange("p (y x) -> p y x", y=32, x=32),
            )

        def xview(j, half, ky, kx):
            v = xp[:, j, :].rearrange("p (y x) -> p y x", y=34, x=34)
            return v[:, half * 16 + ky: half * 16 + ky + 16, kx: kx + 32]

        # ---- epilogue: conv3x3 ob0 (hides last division chain), attnconv,
        # conv3x3 ob1 (its matmuls overlap the attn output moves + DMA) ----
        oconv = ap_.tile([128, 2, HW], f32)
        oattn = ap_.tile([128, 2, HW], f32)

        def conv3_ob(ob):
            ps = pbig.tile([128, HW], f32, tag="big", name=f"convps{ob}")
            for half in range(2):
                for j in range(2):
                    for t in range(9):
                        ky, kx = t // 3, t % 3
                        nc.tensor.matmul(
                            ps[:, half * 512:(half + 1) * 512],
                            wout[:, j, t, ob * 128:(ob + 1) * 128],
                            xview(j, half, ky, kx),
                            start=((j, t) == (0, 0)), stop=((j, t) == (1, 8)),
                        )
            if ob == 0:
                nc.vector.tensor_scalar_add(oconv[:, ob, :], ps[:],
                                            bout[:, ob:ob + 1])
            else:
                nc.scalar.activation(oconv[:, ob, :], ps[:], AF.Identity,
                                     bias=bout[:, ob:ob + 1])
            for hh in range(2):
                nc.sync.dma_start(out_d[ob, :, hh * 512:(hh + 1) * 512],
                                  oconv[:, ob, hh * 512:(hh + 1) * 512])

        def attnconv_ob(ob):
            ps = pbig.tile([128, HW], f32, tag="big", name=f"attnps{ob}")
            for lh in range(2):
                for j in range(2):
                    nc.tensor.matmul(
                        ps[:, lh * 512:(lh + 1) * 512],
                        wattn[:, j, ob * 128:(ob + 1) * 128],
                        attn[:, j, lh * 512:(lh + 1) * 512],
                        start=(j == 0), stop=(j == 1),
                    )
            nc.vector.tensor_scalar_add(oattn[:, ob, :], ps[:],
                                        batt[:, ob:ob + 1])
            for hh in range(2):
                nc.scalar.dma_start(out_d[2 + ob, :, hh * 512:(hh + 1) * 512],
                                    oattn[:, ob, hh * 512:(hh + 1) * 512])

        conv3_ob(0)
        attnconv_ob(0)
        attnconv_ob(1)
        conv3_ob(1)

    nc.compile()
    return nc


def _host_inputs(x, w_qkv, b_qkv, w_attn, b_attn, w_out, b_out,
                 key_rel_w, key_rel_h):
    bf = ml_dtypes.bfloat16
    s = DKH ** -0.5
    wq = np.asarray(w_qkv, np.float32)[:, :, 0, 0].copy()   # [768, 256]
    bq = np.asarray(b_qkv, np.float32).copy()
    wq[:DK] *= s
    bq[:DK] *= s
    wqkvT = np.ascontiguousarray(wq.T).reshape(2, 128, 768).astype(bf)
    wa = np.asarray(w_attn, np.float32)[:, :, 0, 0]          # [256, 256]
    wattnT = np.ascontiguousarray(wa.T).reshape(2, 128, 256).astype(bf)
    woutT = np.ascontiguousarray(
        np.asarray(w_out, np.float32).transpose(1, 2, 3, 0).reshape(256, 9, 256)
    ).reshape(2, 128, 9, 256).astype(bf)

    # block-diagonal shifted windows: diag[32i+d, y, 32i+y2] = krX[31+y2-y, d]
    def diag_windows(kr):
        krT = np.ascontiguousarray(np.asarray(kr, np.float32).T)  # [32, 63]
        idx = 31 + np.arange(32)[None, :] - np.arange(32)[:, None]  # [y, y2]
        base = krT[:, idx]                                   # [32d, 32y, 32y2]
        A = np.zeros((4, 32, 32, 4, 32), np.float32)
        for i in range(4):
            A[i, :, :, i, :] = base
        return np.ascontiguousarray(A.reshape(128, 32, 128)).astype(bf)

    krhdiag = diag_windows(key_rel_h)
    krwdiag = diag_windows(key_rel_w)

    masks = np.zeros((64, 8, 128), np.float32)
    for mb in range(8):
        for jj in range(128):
            masks[(mb * 4 + jj // 32) % 32, mb, jj] = 1.0  # U32 (y2 rows 0:32)
    for jj in range(128):
        masks[32 + jj % 32, :, jj] = 1.0                   # I32 (x2 rows 32:64)
    masks = masks.astype(bf)

    bqkv = np.ascontiguousarray(bq[:512].reshape(4, 128).T)           # [128, 4]
    bv = np.asarray(b_qkv, np.float32)[512:768]
    battn = np.asarray(b_attn, np.float32) + wa @ bv       # fold v-bias
    battn = np.ascontiguousarray(battn.reshape(2, 128).T)
    boutm = np.ascontiguousarray(np.asarray(b_out, np.float32).reshape(2, 128).T)

    shared = dict(wqkvT=wqkvT, wattnT=wattnT, woutT=woutT, krhdiag=krhdiag,
                  krwdiag=krwdiag, masks=masks, bqkv=bqkv, battn=battn,
                  bout=boutm)
    xs = np.asarray(x, np.float32).reshape(B, 2, 128, HW)
    return [dict(shared, x=np.ascontiguousarray(xs[i])) for i in range(N_CORES)]


def kernel(**inputs):
    from concourse.bass_utils import run_bass_kernel_spmd
    if "nc" not in _CACHE:
        _CACHE["nc"] = _build()
    nc = _CACHE["nc"]
    in_maps = _host_inputs(**inputs)
    res = run_bass_kernel_spmd(nc, in_maps, list(range(N_CORES)),
                               trace=bool(os.environ.get("BASS_KERNEL_TRACE")))
    _CACHE["last_result"] = res
    outs = [r["out"].reshape(C_OUT, H, W) for r in res.results]
    return np.stack(outs).astype(np.float32)


# revision 35
# speedup vs baseline: 1.0186x; 1.0186x over previous
"""AttentionConv2d Trainium2 kernel — 8-core batch-data-parallel (v3).

Each of the 8 NeuronCores processes one image of the batch:
  - qkv 1x1 conv (q,k only) + 3x3 conv as implicit-GEMM matmuls
  - V projection computed transposed (x as stationary) so the A*V stationary
    needs no on-device transposes
  - relative position logits G[y2,l]/W[x2,l] produced by 64 diagonal-packed
    matmuls (4 heads x 2 j-halves per pass via block-diagonal kr windows
    built on host)
  - per-head logits via contract-dim augmentation (32 k + 32 G-sel + 32 W-sel
    rows); logits/AV matmuls interleaved mb-wise to keep PE busy (p-state)
  - softmax exp on ScalarE from PSUM; denominators ride the A*V matmul as an
    appended ones-column; fast approx reciprocal + deferred division pipeline
  - final 1x1 conv; outputs concatenated [conv_out(256) ; attn(256)]
  - padded-image build + 3x3 conv run off the attention critical path
"""

import os
import sys

import numpy as np
import ml_dtypes

sys.path.insert(0, "/opt/trn_rl_repo")

B, C_IN, H, W = 8, 256, 32, 32
HW = H * W
DK = DV = 256
NH = 8
DKH = DK // NH  # 32
C_OUT = 512
N_CORES = 8

_CACHE = {}


def _build():
    import concourse.bass as bass
    import concourse.mybir as mybir
    import concourse.tile as tile
    from concourse import bacc
    from contextlib import ExitStack

    f32 = mybir.dt.float32
    bf16 = mybir.dt.bfloat16
    AF = mybir.ActivationFunctionType

    nc = bacc.Bacc("TRN2", target_bir_lowering=False, debug=False,
                   num_devices=N_CORES)

    x_d = nc.dram_tensor("x", [2, 128, HW], f32, kind="ExternalInput").ap()
    wqkv_d = nc.dram_tensor("wqkvT", [2, 128, 768], bf16, kind="ExternalInput").ap()
    wout_d = nc.dram_tensor("woutT", [2, 128, 9, 256], bf16, kind="ExternalInput").ap()
    wattn_d = nc.dram_tensor("wattnT", [2, 128, 256], bf16, kind="ExternalInput").ap()
    khd_d = nc.dram_tensor("krhdiag", [128, 32, 128], bf16, kind="ExternalInput").ap()
    kwd_d = nc.dram_tensor("krwdiag", [128, 32, 128], bf16, kind="ExternalInput").ap()
    masks_d = nc.dram_tensor("masks", [64, 8, 128], bf16, kind="ExternalInput").ap()
    bqkv_d = nc.dram_tensor("bqkv", [128, 4], f32, kind="ExternalInput").ap()
    batt_d = nc.dram_tensor("battn", [128, 2], f32, kind="ExternalInput").ap()
    bout_d = nc.dram_tensor("bout", [128, 2], f32, kind="ExternalInput").ap()
    out_d = nc.dram_tensor("out", [4, 128, HW], f32, kind="ExternalOutput").ap()

    with tile.TileContext(nc) as tc, ExitStack() as ctx:
        wp = ctx.enter_context(tc.tile_pool(name="weights", bufs=1))
        ap_ = ctx.enter_context(tc.tile_pool(name="acts", bufs=1))
        hp = ctx.enter_context(tc.tile_pool(name="head", bufs=2))
        pbig = ctx.enter_context(tc.tile_pool(name="pbig", bufs=2, space="PSUM"))

        # ---- weights / constants to SBUF ----
        wqkv = wp.tile([128, 2, 768], bf16)
        wout = wp.tile([128, 2, 9, 256], bf16)
        wattn = wp.tile([128, 2, 256], bf16)
        khd = wp.tile([128, 32, 128], bf16)
        kwd = wp.tile([128, 32, 128], bf16)
        tmpl = wp.tile([96, 2, 8, 128], bf16)  # [contract, k-slot, mb, jj]
        bqkv = wp.tile([128, 4], f32)
        batt = wp.tile([128, 2], f32)
        bout = wp.tile([128, 2], f32)

        # ---- x first (critical path); fan DMA issue across idle engines ----
        x32 = ap_.tile([128, 2, HW], f32)
        xc = ap_.tile([128, 2, HW], bf16)   # contiguous bf16 image
        xeng = [nc.sync, nc.scalar]
        for j in range(2):
            for hh in range(2):
                xeng[j].dma_start(x32[:, j, hh * 512:(hh + 1) * 512],
                                  x_d[j, :, hh * 512:(hh + 1) * 512])
        nc.sync.dma_start(bqkv[:], bqkv_d[:])
        for j in range(2):
            xeng[j].dma_start(wqkv[:, j, :], wqkv_d[j])
        for hh in range(2):
            nc.scalar.dma_start(khd[:, 16 * hh:16 * hh + 16, :],
                                khd_d[:, 16 * hh:16 * hh + 16, :])
            nc.scalar.dma_start(kwd[:, 16 * hh:16 * hh + 16, :],
                                kwd_d[:, 16 * hh:16 * hh + 16, :])
        for s in range(2):
            nc.gpsimd.dma_start(tmpl[32:96, s, :, :], masks_d[:])
        ceng = [nc.vector, nc.gpsimd]
        for hh in range(2):
            for j in range(2):
                ceng[j].tensor_copy(
                    xc[:, j, hh * 512:(hh + 1) * 512],
                    x32[:, j, hh * 512:(hh + 1) * 512])
        for j in range(2):
            nc.sync.dma_start(wattn[:, j, :], wattn_d[j])
            nc.sync.dma_start(wout[:, j, :, :], wout_d[j])
        nc.sync.dma_start(batt[:], batt_d[:])
        nc.sync.dma_start(bout[:], bout_d[:])

        # ---- qkv = Wqkv @ x (1x1 conv), q then k; vT interleaved ----
        qkv = ap_.tile([128, 4, 32, 32], bf16)
        vTe = ap_.tile([128, 8, 8, 33], bf16)  # [m, mb, h, d(+ones)]
        nc.gpsimd.memset(vTe[:], 1.0)

        def qkv_ob(ob):
            ps = pbig.tile([128, HW], f32, tag="big", name=f"qkvps{ob}")
            for half in range(2):
                for j in range(2):
                    nc.tensor.matmul(
                        ps[:, half * 512:(half + 1) * 512],
                        wqkv[:, j, ob * 128:(ob + 1) * 128],
                        xc[:, j, half * 512:(half + 1) * 512],
                        start=(j == 0), stop=(j == 1),
                    )
            dst = qkv[:, ob, :, :].rearrange("p y x -> p (y x)")
            if ob < 2:
                nc.vector.tensor_scalar_add(dst, ps[:], bqkv[:, ob:ob + 1])
            else:
                nc.scalar.activation(dst, ps[:], AF.Identity,
                                     bias=bqkv[:, ob:ob + 1])

        def vT_half(hb):  # m-blocks 4*hb .. 4*hb+3
            pv = pbig.tile([128, HW], f32, tag="big")
            for bb in range(4):
                b = 4 * hb + bb
                for j in range(2):
                    nc.tensor.matmul(
                        pv[:, bb * 256:(bb + 1) * 256],
                        xc[:, j, 128 * b:128 * (b + 1)],
                        wqkv[:, j, 512:768],
                        start=(j == 0), stop=(j == 1),
                    )
            nc.vector.tensor_copy(
                vTe[:, 4 * hb:4 * hb + 4, :, 0:32],
                pv.rearrange("p (b h d) -> p b h d", b=4, h=8, d=32),
            )

        qkv_ob(0)
        qkv_ob(1)
        vT_half(0)
        vT_half(1)
        qkv_ob(2)
        qkv_ob(3)

        # ---- rel-position logits, 4 heads x 2 j per pass ----
        # G[32i+y2, j, y, x] = sum_d krh[31+y2-y, d] * q[32i+d, j, y, x]
        grelB = ap_.tile([128, 2, HW], bf16)
        wrelB = ap_.tile([128, 2, HW], bf16)
        with tc.tile_pool(name="pmx", bufs=1, space="PSUM") as pmx:
            pg = pmx.tile([128, 2048], f32)
            for y in range(32):
                nc.tensor.matmul(
                    pg[:, y * 64:(y + 1) * 64], khd[:, y, :], qkv[:, 0:2, y, :],
                    start=True, stop=True,
                )
            # pg cols are (y, j, x) -> reorder to (j, y, x) during staging
            nc.scalar.activation(
                grelB[:],
                pg.rearrange("p (y j x) -> p j y x", y=32, j=2, x=32),
                AF.Copy)
            # W-side into pbig tiles so it does not wait on grelB staging
            for xh in range(2):
                pw = pbig.tile([128, HW], f32, tag="big")
                for xx in range(16):
                    x = 16 * xh + xx
                    nc.tensor.matmul(
                        pw[:, xx * 64:(xx + 1) * 64], kwd[:, x, :],
                        qkv[:, 0:2, :, x],
                        start=True, stop=True,
                    )
                # pw cols are (x, j, y) -> (j, y, x)
                nc.vector.tensor_copy(
                    wrelB[:].rearrange("p j (y x) -> p j y x", y=32, x=32)
                    [:, :, :, 16 * xh:16 * xh + 16],
                    pw.rearrange("p (x j y) -> p j y x", x=16, j=2, y=32),
                )

        # ---- per-head attention ----
        pav = ctx.enter_context(tc.tile_pool(name="pav", bufs=2, space="PSUM"))
        attn = ap_.tile([128, 2, HW], bf16)
        pend = []

        def divide(avp, hp0, j):
            dn = hp.tile([1, HW], f32, tag="dn")
            nc.vector.tensor_copy(dn[:], avp[32:33, :])
            rdn = hp.tile([1, HW], f32, tag="rdn")
            nc.vector.reciprocal_approx_fast(rdn[:], dn[:])
            rb = hp.tile([32, HW], f32, tag="rb")
            nc.gpsimd.partition_broadcast(rb[:], rdn[:])
            nc.vector.tensor_mul(attn[hp0:hp0 + 32, j, :], avp[0:32, :], rb[:])

        def av_mb(ctx_h, mb):
            st, avp, hh = ctx_h
            for lh in range(2):
                nc.tensor.matmul(
                    avp[:, lh * 512:(lh + 1) * 512],
                    vTe[:, mb, hh, :],
                    st[:, mb, lh * 512:(lh + 1) * 512],
                    start=(mb == 0), stop=(mb == 7),
                )

        prev = None  # (st, avp, h) with A6/A7 still to issue
        for h in range(NH):
            i, j, s = h % 4, h // 4, h % 2
            hp0 = 32 * i
            nc.vector.tensor_copy(
                tmpl[0:32, s, :, :],
                qkv[hp0:hp0 + 32, 2 + j, :, :].rearrange("p y x -> p (y x)")
                .rearrange("p (m c) -> p m c", m=8, c=128),
            )
            rhs = hp.tile([96, HW], bf16, tag="rhs")
            nc.vector.tensor_copy(
                rhs[0:32, :],
                qkv[hp0:hp0 + 32, j, :, :].rearrange("p y x -> p (y x)"))
            nc.vector.tensor_copy(rhs[32:64, :], grelB[hp0:hp0 + 32, j, :])
            nc.vector.tensor_copy(rhs[64:96, :], wrelB[hp0:hp0 + 32, j, :])

            st = hp.tile([128, 8, HW], bf16, tag="st")
            avp = pav.tile([33, HW], f32, tag="av")
            cur = (st, avp, h)

            def logits_mb(mb):
                ps = pbig.tile([128, HW], f32, tag="big")
                for lh in range(2):
                    nc.tensor.matmul(
                        ps[:, lh * 512:(lh + 1) * 512],
                        tmpl[:, s, mb, :],
                        rhs[:, lh * 512:(lh + 1) * 512],
                        start=True, stop=True,
                    )
                nc.scalar.activation(st[:, mb, :], ps[:], AF.Exp)

            # cross-head pipeline: finish the previous head's tail A*V inside
            # this head's slot stream, then its division, so PE never drains
            logits_mb(0)
            logits_mb(1)
            if prev is not None:
                av_mb(prev, 6)
                av_mb(prev, 7)
                divide(*pend.pop())
            for mb in range(2, 8):
                av_mb(cur, mb - 2)
                logits_mb(mb)
            pend.append((avp, hp0, j))
            prev = cur

        av_mb(prev, 6)
        av_mb(prev, 7)
        if pend:
            divide(*pend.pop())

        # ---- padded image for 3x3 conv (off critical path) ----
        xp = ap_.tile([128, 2, 34 * 34], bf16)
        nc.gpsimd.memset(xp[:], 0.0)
        for j in range(2):
            nc.gpsimd.tensor_copy(
                xp[:, j, :].rearrange("p (y x) -> p y x", y=34, x=34)[:, 1:33, 1:33],
                xc[:, j, :].rearrange("p (y x) -> p y x", y=32, x=32),
            )

        def xview(j, half, ky, kx):
            v = xp[:, j, :].rearrange("p (y x) -> p y x", y=34, x=34)
            return v[:, half * 16 + ky: half * 16 + ky + 16, kx: kx + 32]

        # ---- epilogue: conv3x3 ob0 (hides last division chain), attnconv,
        # conv3x3 ob1 (its matmuls overlap the attn output moves + DMA) ----
        oconv = ap_.tile([128, 2, HW], f32)
        oattn = ap_.tile([128, 2, HW], f32)

        def conv3_ob(ob):
            ps = pbig.tile([128, HW], f32, tag="big", name=f"convps{ob}")
            for half in range(2):
                for j in range(2):
                    for t in range(9):
                        ky, kx = t // 3, t % 3
                        nc.tensor.matmul(
                            ps[:, half * 512:(half + 1) * 512],
                            wout[:, j, t, ob * 128:(ob + 1) * 128],
                            xview(j, half, ky, kx),
                            start=((j, t) == (0, 0)), stop=((j, t) == (1, 8)),
                        )
            if ob == 0:
                nc.vector.tensor_scalar_add(oconv[:, ob, :], ps[:],
                                            bout[:, ob:ob + 1])
            else:
                nc.scalar.activation(oconv[:, ob, :], ps[:], AF.Identity,
                                     bias=bout[:, ob:ob + 1])
            for hh in range(2):
                nc.sync.dma_start(out_d[ob, :, hh * 512:(hh + 1) * 512],
                                  oconv[:, ob, hh * 512:(hh + 1) * 512])

        def attnconv_ob(ob):
            ps = pbig.tile([128, HW], f32, tag="big", name=f"attnps{ob}")
            for lh in range(2):
                for j in range(2):
                    nc.tensor.matmul(
                        ps[:, lh * 512:(lh + 1) * 512],
                        wattn[:, j, ob * 128:(ob + 1) * 128],
                        attn[:, j, lh * 512:(lh + 1) * 512],
                        start=(j == 0), stop=(j == 1),
                    )
            nc.vector.tensor_scalar_add(oattn[:, ob, :], ps[:],
                                        batt[:, ob:ob + 1])
            for hh in range(2):
                nc.scalar.dma_start(out_d[2 + ob, :, hh * 512:(hh + 1) * 512],
                                    oattn[:, ob, hh * 512:(hh + 1) * 512])

        conv3_ob(0)
        attnconv_ob(0)
        attnconv_ob(1)
        conv3_ob(1)

    nc.compile()
    return nc


def _host_inputs(x, w_qkv, b_qkv, w_attn, b_attn, w_out, b_out,
                 key_rel_w, key_rel_h):
    bf = ml_dtypes.bfloat16
    s = DKH ** -0.5
    wq = np.asarray(w_qkv, np.float32)[:, :, 0, 0].copy()   # [768, 256]
    bq = np.asarray(b_qkv, np.float32).copy()
    wq[:DK] *= s
    bq[:DK] *= s
    wqkvT = np.ascontiguousarray(wq.T).reshape(2, 128, 768).astype(bf)
    wa = np.asarray(w_attn, np.float32)[:, :, 0, 0]          # [256, 256]
    wattnT = np.ascontiguousarray(wa.T).reshape(2, 128, 256).astype(bf)
    woutT = np.ascontiguousarray(
        np.asarray(w_out, np.float32).transpose(1, 2, 3, 0).reshape(256, 9, 256)
    ).reshape(2, 128, 9, 256).astype(bf)

    # block-diagonal shifted windows: diag[32i+d, y, 32i+y2] = krX[31+y2-y, d]
    def diag_windows(kr):
        krT = np.ascontiguousarray(np.asarray(kr, np.float32).T)  # [32, 63]
        idx = 31 + np.arange(32)[None, :] - np.arange(32)[:, None]  # [y, y2]
        base = krT[:, idx]                                   # [32d, 32y, 32y2]
        A = np.zeros((4, 32, 32, 4, 32), np.float32)
        for i in range(4):
            A[i, :, :, i, :] = base
        return np.ascontiguousarray(A.reshape(128, 32, 128)).astype(bf)

    krhdiag = diag_windows(key_rel_h)
    krwdiag = diag_windows(key_rel_w)

    masks = np.zeros((64, 8, 128), np.float32)
    for mb in range(8):
        for jj in range(128):
            masks[(mb * 4 + jj // 32) % 32, mb, jj] = 1.0  # U32 (y2 rows 0:32)
    for jj in range(128):
        masks[32 + jj % 32, :, jj] = 1.0                   # I32 (x2 rows 32:64)
    masks = masks.astype(bf)

    bqkv = np.ascontiguousarray(bq[:512].reshape(4, 128).T)           # [128, 4]
    bv = np.asarray(b_qkv, np.float32)[512:768]
    battn = np.asarray(b_attn, np.float32) + wa @ bv       # fold v-bias
    battn = np.ascontiguousarray(battn.reshape(2, 128).T)
    boutm = np.ascontiguousarray(np.asarray(b_out, np.float32).reshape(2, 128).T)

    shared = dict(wqkvT=wqkvT, wattnT=wattnT, woutT=woutT, krhdiag=krhdiag,
                  krwdiag=krwdiag, masks=masks, bqkv=bqkv, battn=battn,
                  bout=boutm)
    xs = np.asarray(x, np.float32).reshape(B, 2, 128, HW)
    return [dict(shared, x=np.ascontiguousarray(xs[i])) for i in range(N_CORES)]


def kernel(**inputs):
    from concourse.bass_utils import run_bass_kernel_spmd
    if "nc" not in _CACHE:
        _CACHE["nc"] = _build()
    nc = _CACHE["nc"]
    in_maps = _host_inputs(**inputs)
    res = run_bass_kernel_spmd(nc, in_maps, list(range(N_CORES)),
                               trace=bool(os.environ.get("BASS_KERNEL_TRACE")))
    _CACHE["last_result"] = res
    outs = [r["out"].reshape(C_OUT, H, W) for r in res.results]
    return np.stack(outs).astype(np.float32)


# revision 36
# speedup vs baseline: 1.0221x; 1.0035x over previous
"""AttentionConv2d Trainium2 kernel — 8-core batch-data-parallel (v3).

Each of the 8 NeuronCores processes one image of the batch:
  - qkv 1x1 conv (q,k only) + 3x3 conv as implicit-GEMM matmuls
  - V projection computed transposed (x as stationary) so the A*V stationary
    needs no on-device transposes
  - relative position logits G[y2,l]/W[x2,l] produced by 64 diagonal-packed
    matmuls (4 heads x 2 j-halves per pass via block-diagonal kr windows
    built on host)
  - per-head logits via contract-dim augmentation (32 k + 32 G-sel + 32 W-sel
    rows); logits/AV matmuls interleaved mb-wise to keep PE busy (p-state)
  - softmax exp on ScalarE from PSUM; denominators ride the A*V matmul as an
    appended ones-column; fast approx reciprocal + deferred division pipeline
  - final 1x1 conv; outputs concatenated [conv_out(256) ; attn(256)]
  - padded-image build + 3x3 conv run off the attention critical path
"""

import os
import sys

import numpy as np
import ml_dtypes

sys.path.insert(0, "/opt/trn_rl_repo")

B, C_IN, H, W = 8, 256, 32, 32
HW = H * W
DK = DV = 256
NH = 8
DKH = DK // NH  # 32
C_OUT = 512
N_CORES = 8

_CACHE = {}


def _build():
    import concourse.bass as bass
    import concourse.mybir as mybir
    import concourse.tile as tile
    from concourse import bacc
    from contextlib import ExitStack

    f32 = mybir.dt.float32
    bf16 = mybir.dt.bfloat16
    AF = mybir.ActivationFunctionType

    nc = bacc.Bacc("TRN2", target_bir_lowering=False, debug=False,
                   num_devices=N_CORES)

    x_d = nc.dram_tensor("x", [2, 128, HW], f32, kind="ExternalInput").ap()
    wqkv_d = nc.dram_tensor("wqkvT", [2, 128, 768], bf16, kind="ExternalInput").ap()
    wout_d = nc.dram_tensor("woutT", [2, 128, 9, 256], bf16, kind="ExternalInput").ap()
    wattn_d = nc.dram_tensor("wattnT", [2, 128, 256], bf16, kind="ExternalInput").ap()
    khd_d = nc.dram_tensor("krhdiag", [128, 32, 128], bf16, kind="ExternalInput").ap()
    kwd_d = nc.dram_tensor("krwdiag", [128, 32, 128], bf16, kind="ExternalInput").ap()
    masks_d = nc.dram_tensor("masks", [64, 8, 128], bf16, kind="ExternalInput").ap()
    bqkv_d = nc.dram_tensor("bqkv", [128, 4], f32, kind="ExternalInput").ap()
    batt_d = nc.dram_tensor("battn", [128, 2], f32, kind="ExternalInput").ap()
    bout_d = nc.dram_tensor("bout", [128, 2], f32, kind="ExternalInput").ap()
    out_d = nc.dram_tensor("out", [4, 128, HW], f32, kind="ExternalOutput").ap()

    with tile.TileContext(nc) as tc, ExitStack() as ctx:
        wp = ctx.enter_context(tc.tile_pool(name="weights", bufs=1))
        ap_ = ctx.enter_context(tc.tile_pool(name="acts", bufs=1))
        hp = ctx.enter_context(tc.tile_pool(name="head", bufs=2))
        pbig = ctx.enter_context(tc.tile_pool(name="pbig", bufs=2, space="PSUM"))

        # ---- weights / constants to SBUF ----
        wqkv = wp.tile([128, 2, 768], bf16)
        wout = wp.tile([128, 2, 9, 256], bf16)
        wattn = wp.tile([128, 2, 256], bf16)
        khd = wp.tile([128, 32, 128], bf16)
        kwd = wp.tile([128, 32, 128], bf16)
        tmpl = wp.tile([96, 2, 8, 128], bf16)  # [contract, k-slot, mb, jj]
        bqkv = wp.tile([128, 4], f32)
        batt = wp.tile([128, 2], f32)
        bout = wp.tile([128, 2], f32)

        # ---- x first (critical path); fan DMA issue across idle engines ----
        x32 = ap_.tile([128, 2, HW], f32)
        xc = ap_.tile([128, 2, HW], bf16)   # contiguous bf16 image
        xeng = [nc.sync, nc.scalar]
        for j in range(2):
            for hh in range(2):
                xeng[j].dma_start(x32[:, j, hh * 512:(hh + 1) * 512],
                                  x_d[j, :, hh * 512:(hh + 1) * 512])
        nc.sync.dma_start(bqkv[:], bqkv_d[:])
        for j in range(2):
            xeng[j].dma_start(wqkv[:, j, :], wqkv_d[j])
        for hh in range(2):
            nc.scalar.dma_start(khd[:, 16 * hh:16 * hh + 16, :],
                                khd_d[:, 16 * hh:16 * hh + 16, :])
            nc.scalar.dma_start(kwd[:, 16 * hh:16 * hh + 16, :],
                                kwd_d[:, 16 * hh:16 * hh + 16, :])
        for s in range(2):
            nc.gpsimd.dma_start(tmpl[32:96, s, :, :], masks_d[:])
        ceng = [nc.vector, nc.gpsimd]
        for hh in range(2):
            for j in range(2):
                ceng[j].tensor_copy(
                    xc[:, j, hh * 512:(hh + 1) * 512],
                    x32[:, j, hh * 512:(hh + 1) * 512])
        for j in range(2):
            nc.sync.dma_start(wattn[:, j, :], wattn_d[j])
            nc.sync.dma_start(wout[:, j, :, :], wout_d[j])
        nc.sync.dma_start(batt[:], batt_d[:])
        nc.sync.dma_start(bout[:], bout_d[:])

        # ---- qkv = Wqkv @ x (1x1 conv), q then k; vT interleaved ----
        qkv = ap_.tile([128, 4, 32, 32], bf16)
        vTe = ap_.tile([128, 8, 8, 33], bf16)  # [m, mb, h, d(+ones)]
        nc.gpsimd.memset(vTe[:], 1.0)

        def qkv_ob(ob):
            ps = pbig.tile([128, HW], f32, tag="big", name=f"qkvps{ob}")
            for half in range(2):
                for j in range(2):
                    nc.tensor.matmul(
                        ps[:, half * 512:(half + 1) * 512],
                        wqkv[:, j, ob * 128:(ob + 1) * 128],
                        xc[:, j, half * 512:(half + 1) * 512],
                        start=(j == 0), stop=(j == 1),
                    )
            dst = qkv[:, ob, :, :].rearrange("p y x -> p (y x)")
            if ob < 2:
                nc.vector.tensor_scalar_add(dst, ps[:], bqkv[:, ob:ob + 1])
            else:
                nc.scalar.activation(dst, ps[:], AF.Identity,
                                     bias=bqkv[:, ob:ob + 1])

        def vT_half(hb):  # m-blocks 4*hb .. 4*hb+3
            pv = pbig.tile([128, HW], f32, tag="big")
            for bb in range(4):
                b = 4 * hb + bb
                for j in range(2):
                    nc.tensor.matmul(
                        pv[:, bb * 256:(bb + 1) * 256],
                        xc[:, j, 128 * b:128 * (b + 1)],
                        wqkv[:, j, 512:768],
                        start=(j == 0), stop=(j == 1),
                    )
            nc.vector.tensor_copy(
                vTe[:, 4 * hb:4 * hb + 4, :, 0:32],
                pv.rearrange("p (b h d) -> p b h d", b=4, h=8, d=32),
            )

        qkv_ob(0)
        qkv_ob(1)
        vT_half(0)
        vT_half(1)
        qkv_ob(2)
        qkv_ob(3)

        # ---- rel-position logits, 4 heads x 2 j per pass ----
        # G[32i+y2, j, y, x] = sum_d krh[31+y2-y, d] * q[32i+d, j, y, x]
        grelB = ap_.tile([128, 2, HW], bf16)
        wrelB = ap_.tile([128, 2, HW], bf16)
        with tc.tile_pool(name="pmx", bufs=1, space="PSUM") as pmx:
            pg = pmx.tile([128, 2048], f32)
            for y in range(32):
                nc.tensor.matmul(
                    pg[:, y * 64:(y + 1) * 64], khd[:, y, :], qkv[:, 0:2, y, :],
                    start=True, stop=True,
                )
            # pg cols are (y, j, x) -> reorder to (j, y, x) during staging
            nc.scalar.activation(
                grelB[:],
                pg.rearrange("p (y j x) -> p j y x", y=32, j=2, x=32),
                AF.Copy)
            # W-side into pbig tiles so it does not wait on grelB staging
            for xh in range(2):
                pw = pbig.tile([128, HW], f32, tag="big")
                for xx in range(16):
                    x = 16 * xh + xx
                    nc.tensor.matmul(
                        pw[:, xx * 64:(xx + 1) * 64], kwd[:, x, :],
                        qkv[:, 0:2, :, x],
                        start=True, stop=True,
                    )
                # pw cols are (x, j, y) -> (j, y, x)
                nc.vector.tensor_copy(
                    wrelB[:].rearrange("p j (y x) -> p j y x", y=32, x=32)
                    [:, :, :, 16 * xh:16 * xh + 16],
                    pw.rearrange("p (x j y) -> p j y x", x=16, j=2, y=32),
                )

        # ---- per-head attention ----
        pav = ctx.enter_context(tc.tile_pool(name="pav", bufs=2, space="PSUM"))
        attn = ap_.tile([128, 2, HW], bf16)
        pend = []

        def divide(avp, hp0, j):
            dn = hp.tile([1, HW], f32, tag="dn")
            nc.vector.tensor_copy(dn[:], avp[32:33, :])
            rdn = hp.tile([1, HW], f32, tag="rdn")
            nc.vector.reciprocal_approx_fast(rdn[:], dn[:])
            rb = hp.tile([32, HW], f32, tag="rb")
            nc.gpsimd.partition_broadcast(rb[:], rdn[:])
            nc.vector.tensor_mul(attn[hp0:hp0 + 32, j, :], avp[0:32, :], rb[:])

        def av_mb(ctx_h, mb):
            st, avp, hh = ctx_h
            for lh in range(2):
                nc.tensor.matmul(
                    avp[:, lh * 512:(lh + 1) * 512],
                    vTe[:, mb, hh, :],
                    st[:, mb, lh * 512:(lh + 1) * 512],
                    start=(mb == 0), stop=(mb == 7),
                )

        prev = None  # (st, avp, h) with A6/A7 still to issue
        for h in range(NH):
            i, j, s = h % 4, h // 4, h % 2
            hp0 = 32 * i
            nc.vector.tensor_copy(
                tmpl[0:32, s, :, :],
                qkv[hp0:hp0 + 32, 2 + j, :, :].rearrange("p y x -> p (y x)")
                .rearrange("p (m c) -> p m c", m=8, c=128),
            )
            rhs = hp.tile([96, HW], bf16, tag="rhs")
            nc.vector.tensor_copy(
                rhs[0:32, :],
                qkv[hp0:hp0 + 32, j, :, :].rearrange("p y x -> p (y x)"))
            nc.vector.tensor_copy(rhs[32:64, :], grelB[hp0:hp0 + 32, j, :])
            nc.vector.tensor_copy(rhs[64:96, :], wrelB[hp0:hp0 + 32, j, :])

            st = hp.tile([128, 8, HW], bf16, tag="st")
            avp = pav.tile([33, HW], f32, tag="av")
            cur = (st, avp, h)

            def logits_mb(mb):
                ps = pbig.tile([128, HW], f32, tag="big")
                for lh in range(2):
                    mm = nc.tensor.matmul(
                        ps[:, lh * 512:(lh + 1) * 512],
                        tmpl[:, s, mb, :],
                        rhs[:, lh * 512:(lh + 1) * 512],
                        start=True, stop=True,
                    )
                    if lh == 1:
                        # same stationary as lh=0: skip the redundant reload
                        mm.ins.ldweights = False
                nc.scalar.activation(st[:, mb, :], ps[:], AF.Exp)

            # cross-head pipeline: finish the previous head's tail A*V inside
            # this head's slot stream, then its division, so PE never drains
            logits_mb(0)
            logits_mb(1)
            if prev is not None:
                av_mb(prev, 6)
                av_mb(prev, 7)
                divide(*pend.pop())
            for mb in range(2, 8):
                av_mb(cur, mb - 2)
                logits_mb(mb)
            pend.append((avp, hp0, j))
            prev = cur

        av_mb(prev, 6)
        av_mb(prev, 7)
        if pend:
            divide(*pend.pop())

        # ---- padded image for 3x3 conv (off critical path) ----
        xp = ap_.tile([128, 2, 34 * 34], bf16)
        nc.gpsimd.memset(xp[:], 0.0)
        for j in range(2):
            nc.gpsimd.tensor_copy(
                xp[:, j, :].rearrange("p (y x) -> p y x", y=34, x=34)[:, 1:33, 1:33],
                xc[:, j, :].rearrange("p (y x) -> p y x", y=32, x=32),
            )

        def xview(j, half, ky, kx):
            v = xp[:, j, :].rearrange("p (y x) -> p y x", y=34, x=34)
            return v[:, half * 16 + ky: half * 16 + ky + 16, kx: kx + 32]

        # ---- epilogue: conv3x3 ob0 (hides last division chain), attnconv,
        # conv3x3 ob1 (its matmuls overlap the attn output moves + DMA) ----
        oconv = ap_.tile([128, 2, HW], f32)
        oattn = ap_.tile([128, 2, HW], f32)

        def conv3_ob(ob):
            ps = pbig.tile([128, HW], f32, tag="big", name=f"convps{ob}")
            for half in range(2):
                for j in range(2):
                    for t in range(9):
                        ky, kx = t // 3, t % 3
                        nc.tensor.matmul(
                            ps[:, half * 512:(half + 1) * 512],
                            wout[:, j, t, ob * 128:(ob + 1) * 128],
                            xview(j, half, ky, kx),
                            start=((j, t) == (0, 0)), stop=((j, t) == (1, 8)),
                        )
            if ob == 0:
                nc.vector.tensor_scalar_add(oconv[:, ob, :], ps[:],
                                            bout[:, ob:ob + 1])
            else:
                nc.scalar.activation(oconv[:, ob, :], ps[:], AF.Identity,
                                     bias=bout[:, ob:ob + 1])
            for hh in range(2):
                nc.sync.dma_start(out_d[ob, :, hh * 512:(hh + 1) * 512],
                                  oconv[:, ob, hh * 512:(hh + 1) * 512])

        def attnconv_ob(ob):
            ps = pbig.tile([128, HW], f32, tag="big", name=f"attnps{ob}")
            for lh in range(2):
                for j in range(2):
                    nc.tensor.matmul(
                        ps[:, lh * 512:(lh + 1) * 512],
                        wattn[:, j, ob * 128:(ob + 1) * 128],
                        attn[:, j, lh * 512:(lh + 1) * 512],
                        start=(j == 0), stop=(j == 1),
                    )
            nc.vector.tensor_scalar_add(oattn[:, ob, :], ps[:],
                                        batt[:, ob:ob + 1])
            for hh in range(2):
                nc.scalar.dma_start(out_d[2 + ob, :, hh * 512:(hh + 1) * 512],
                                    oattn[:, ob, hh * 512:(hh + 1) * 512])

        conv3_ob(0)
        attnconv_ob(0)
        attnconv_ob(1)
        conv3_ob(1)

    nc.compile()
    return nc


def _host_inputs(x, w_qkv, b_qkv, w_attn, b_attn, w_out, b_out,
                 key_rel_w, key_rel_h):
    bf = ml_dtypes.bfloat16
    s = DKH ** -0.5
    wq = np.asarray(w_qkv, np.float32)[:, :, 0, 0].copy()   # [768, 256]
    bq = np.asarray(b_qkv, np.float32).copy()
    wq[:DK] *= s
    bq[:DK] *= s
    wqkvT = np.ascontiguousarray(wq.T).reshape(2, 128, 768).astype(bf)
    wa = np.asarray(w_attn, np.float32)[:, :, 0, 0]          # [256, 256]
    wattnT = np.ascontiguousarray(wa.T).reshape(2, 128, 256).astype(bf)
    woutT = np.ascontiguousarray(
        np.asarray(w_out, np.float32).transpose(1, 2, 3, 0).reshape(256, 9, 256)
    ).reshape(2, 128, 9, 256).astype(bf)

    # block-diagonal shifted windows: diag[32i+d, y, 32i+y2] = krX[31+y2-y, d]
    def diag_windows(kr):
        krT = np.ascontiguousarray(np.asarray(kr, np.float32).T)  # [32, 63]
        idx = 31 + np.arange(32)[None, :] - np.arange(32)[:, None]  # [y, y2]
        base = krT[:, idx]                                   # [32d, 32y, 32y2]
        A = np.zeros((4, 32, 32, 4, 32), np.float32)
        for i in range(4):
            A[i, :, :, i, :] = base
        return np.ascontiguousarray(A.reshape(128, 32, 128)).astype(bf)

    krhdiag = diag_windows(key_rel_h)
    krwdiag = diag_windows(key_rel_w)

    masks = np.zeros((64, 8, 128), np.float32)
    for mb in range(8):
        for jj in range(128):
            masks[(mb * 4 + jj // 32) % 32, mb, jj] = 1.0  # U32 (y2 rows 0:32)
    for jj in range(128):
        masks[32 + jj % 32, :, jj] = 1.0                   # I32 (x2 rows 32:64)
    masks = masks.astype(bf)

    bqkv = np.ascontiguousarray(bq[:512].reshape(4, 128).T)           # [128, 4]
    bv = np.asarray(b_qkv, np.float32)[512:768]
    battn = np.asarray(b_attn, np.float32) + wa @ bv       # fold v-bias
    battn = np.ascontiguousarray(battn.reshape(2, 128).T)
    boutm = np.ascontiguousarray(np.asarray(b_out, np.float32).reshape(2, 128).T)

    shared = dict(wqkvT=wqkvT, wattnT=wattnT, woutT=woutT, krhdiag=krhdiag,
                  krwdiag=krwdiag, masks=masks, bqkv=bqkv, battn=battn,
                  bout=boutm)
    xs = np.asarray(x, np.float32).reshape(B, 2, 128, HW)
    return [dict(shared, x=np.ascontiguousarray(xs[i])) for i in range(N_CORES)]


def kernel(**inputs):
    from concourse.bass_utils import run_bass_kernel_spmd
    if "nc" not in _CACHE:
        _CACHE["nc"] = _build()
    nc = _CACHE["nc"]
    in_maps = _host_inputs(**inputs)
    res = run_bass_kernel_spmd(nc, in_maps, list(range(N_CORES)),
                               trace=bool(os.environ.get("BASS_KERNEL_TRACE")))
    _CACHE["last_result"] = res
    outs = [r["out"].reshape(C_OUT, H, W) for r in res.results]
    return np.stack(outs).astype(np.float32)
